# revision 3
# baseline (speedup 1.0000x reference)
"""Trainium2 Bass kernel: 4096x4096 valid cross-correlation with an 11x11
filter + scalar bias, sharded row-wise across 8 NeuronCores.

Strategy
--------
Host-side sharding (halo = overlapping row slices, no collectives): core m
gets input rows [512m, 512m + 522) (core 7 shifted up to stay in bounds)
and produces output rows [512m, 512m + 512).

Per-core compute: conv expressed as banded matmuls on the TensorEngine.
For each kernel column dj, a banded stationary matrix
    B_dj[k, m] = w[k - m, dj]   (0 <= k - m < 11)
contracts over 128 image rows, while column-shifted slices of the image
slab stream as the moving operand:
    out[m, n] += sum_k B_dj[k, m] * x[r0 + k, n0 + n + dj]
Accumulating the 11 dj-shifted matmuls in one PSUM bank yields the full
11x11 correlation for a [118, 512] output tile.

v2 vs the f32r baseline (138.4us):
- bf16 operands: ~217 ns vs 239.5 ns sustained per 512-col matmul on HW
  (less clock throttle), and input DMA halves. Host casts x -> bf16.
- bf16 output + host cast back to f32: store traffic halves.
- slab 0 is loaded in column chunks so bank 0's matmuls start ~4us in
  instead of waiting ~22us for the whole 2MB slab (subtile DMA deps).
- acts+stores are per [M, 512] bank tile, so the tail after the last
  matmul is one act + one 120KB store instead of a whole-slab store.
"""

import os
import sys

import numpy as np

for _p in ("/opt/trn_rl_repo", "/root/.axon_site/_ro/trn_rl_repo"):
    if os.path.isdir(_p) and _p not in sys.path:
        sys.path.insert(0, _p)

_jp = os.environ.get("JAX_PLATFORMS", "")
if "axon" not in _jp.split(","):
    os.environ["JAX_PLATFORMS"] = ("axon," + _jp).strip(",")

import ml_dtypes
import concourse.bacc as bacc
import concourse.bass as bass
import concourse.mybir as mybir
import concourse.tile as tile
from concourse.bass_utils import run_bass_kernel_spmd

H = W = 4096
KH = KW = 11
OH = OW = H - KH + 1  # 4086
NCORES = 8
ROWS_OUT = 512            # output rows per core
ROWS_IN = ROWS_OUT + KH - 1  # 522
M_FULL = 118              # output rows per full slab (contraction K = 128)
# (x row offset, out row offset, M out rows, band column offset) per slab.
# All slabs contract over K=128 input rows: the 40-row tail reads the last
# 128 slab rows (394..521) and picks the shifted band columns 78..117, so
# the PE always runs at full contraction duty.
SLABS = [(0, 0, 118, 0), (118, 118, 118, 0), (236, 236, 118, 0),
         (354, 354, 118, 0), (394, 472, 40, 78)]
BANK_N = [512] * 7 + [OW - 7 * 512]  # 7x512 + 502 = 4086

_cache: dict = {}
LAST_RESULT = None  # BassKernelResults of the most recent device run


def _build():
    f32 = mybir.dt.float32
    bf16 = mybir.dt.bfloat16
    nc = bacc.Bacc("TRN2", target_bir_lowering=False, debug=False,
                   num_devices=NCORES)
    xs_d = nc.dram_tensor("xs", [ROWS_IN, W], bf16, kind="ExternalInput")
    bd_d = nc.dram_tensor("bands", [128, KW * M_FULL], bf16,
                          kind="ExternalInput")
    bias_d = nc.dram_tensor("biasv", [1, 1], f32, kind="ExternalInput")
    out_d = nc.dram_tensor("out", [ROWS_OUT, OW], bf16, kind="ExternalOutput")

    with tile.TileContext(nc) as tc:
        with (
            tc.tile_pool(name="bp", bufs=1) as bp,
            tc.tile_pool(name="xp", bufs=1) as xp,
            tc.tile_pool(name="op", bufs=4) as op,
            tc.tile_pool(name="pp", bufs=6, space=bass.MemorySpace.PSUM) as pp,
            tc.tile_pool(name="pw", bufs=1, space=bass.MemorySpace.PSUM) as pw,
        ):
            # bands first: single DMA, all 16 SDMA engines, ~1us
            bt = bp.tile([128, KW * M_FULL], bf16, name="bt")
            nc.sync.dma_start(bt[:], bd_d.ap()[:, :])

            # slab 0 in column chunks so bank 0 can start as soon as its
            # columns land (Tile subtile deps): [0:522] then 3 wider chunks
            xt0 = xp.tile([128, W], bf16, tag="xt0", name="xt0")
            for c0, c1 in ((0, 522), (522, 1546), (1546, 2570), (2570, W)):
                nc.sync.dma_start(xt0[:, c0:c1], xs_d.ap()[0:128, c0:c1])

            # remaining slabs: whole-tile DMAs (one per slab, 16 engines)
            xts = {0: xt0}
            for si, (r0, _, _, _) in enumerate(SLABS):
                if si == 0:
                    continue
                xt = xp.tile([128, W], bf16, tag=f"xt{si}", name=f"xt{si}")
                nc.sync.dma_start(xt[:], xs_d.ap()[r0:r0 + 128, :])
                xts[si] = xt

            # bias: one-packet DMA, then broadcast across partitions with a
            # K=1 matmul against a ones row
            bias_sb = bp.tile([1, 1], f32, name="bias_sb")
            nc.sync.dma_start(bias_sb[:], bias_d.ap()[:, :])

            # warm the PE pstate while DMAs land; ones_t doubles as the
            # bias-broadcast stationary
            ones_t = bp.tile([1, 128], f32, name="ones_t")
            nc.gpsimd.memset(ones_t[:], 1.0)
            warm_src = bp.tile([128, 512], bf16, name="warm_src")
            nc.gpsimd.memset(warm_src[:], 1.0)
            warm = pw.tile([118, 512], f32, name="warm")
            for i in range(9):
                nc.tensor.matmul(warm[:, :], warm_src[:, 0:118],
                                 warm_src[:, 0:512],
                                 start=(i == 0), stop=(i == 8))
            bias_ps = pw.tile([128, 1], f32, name="bias_ps")
            nc.tensor.matmul(bias_ps[:], ones_t[:], bias_sb[:],
                             start=True, stop=True)
            bias_bc = bp.tile([128, 1], f32, name="bias_bc")
            nc.scalar.copy(bias_bc[:], bias_ps[:])

            for si, (r0, o0, M, boff) in enumerate(SLABS):
                xt = xts[si]
                for b in range(8):
                    n0 = b * 512
                    N = BANK_N[b]
                    pt = pp.tile([M, 512], f32, tag="ps", name=f"ps{si}_{b}")
                    for dj in range(KW):
                        nc.tensor.matmul(
                            pt[:, :N],
                            bt[:, dj * M_FULL + boff: dj * M_FULL + boff + M],
                            xt[:, n0 + dj: n0 + dj + N],
                            start=(dj == 0),
                            stop=(dj == KW - 1),
                        )
                    ot = op.tile([M, 512], bf16, tag="ot", name=f"ot{si}_{b}")
                    nc.scalar.activation(
                        ot[:, :N], pt[:, :N],
                        mybir.ActivationFunctionType.Identity,
                        bias=bias_bc[0:M, :],
                    )
                    nc.sync.dma_start(out_d.ap()[o0:o0 + M, n0:n0 + N],
                                      ot[:, :N])
    nc.compile()
    return nc


def _bands_from_weight(weight: np.ndarray) -> np.ndarray:
    b = np.zeros((128, KW * M_FULL), np.float32)
    for dj in range(KW):
        col = weight[:, dj].astype(np.float32)
        for m in range(M_FULL):
            b[m:m + KH, dj * M_FULL + m] = col
    return b


def kernel(x: np.ndarray, weight: np.ndarray, bias: np.ndarray,
           _trace: bool = False, **_trace_kwargs) -> np.ndarray:
    global LAST_RESULT
    x = np.asarray(x, dtype=np.float32)
    weight = np.asarray(weight, dtype=np.float32)
    bias_v = np.asarray(bias, dtype=np.float32).reshape(1, 1)

    if "nc" not in _cache:
        _cache["nc"] = _build()
    nc = _cache["nc"]

    xb = x.astype(ml_dtypes.bfloat16)
    bands = _bands_from_weight(weight).astype(ml_dtypes.bfloat16)
    starts = [min(m * ROWS_OUT, H - ROWS_IN) for m in range(NCORES)]
    in_maps = [
        {"xs": np.ascontiguousarray(xb[s:s + ROWS_IN]),
         "bands": bands,
         "biasv": bias_v}
        for s in starts
    ]
    res = run_bass_kernel_spmd(nc, in_maps, core_ids=list(range(NCORES)),
                               trace=_trace, **_trace_kwargs)
    LAST_RESULT = res

    out = np.empty((OH, OW), dtype=np.float32)
    for m, s in enumerate(starts):
        r = np.asarray(res.results[m]["out"], dtype=np.float32)
        g0 = m * ROWS_OUT           # first global output row wanted from core m
        keep0 = g0 - s              # 0 for cores 0-6, 10 for core 7
        take = min(ROWS_OUT - keep0, OH - g0)
        out[g0:g0 + take] = r[keep0:keep0 + take]
    return out


# revision 5
# speedup vs baseline: 1.0140x; 1.0140x over previous
"""Trainium2 Bass kernel: 4096x4096 valid cross-correlation with an 11x11
filter + scalar bias, sharded row-wise across 8 NeuronCores.

Strategy
--------
Host-side sharding (halo = overlapping row slices, no collectives): core m
gets input rows [512m, 512m + 522) (core 7 shifted up to stay in bounds)
and produces output rows [512m, 512m + 512).

Per-core compute: conv expressed as banded matmuls on the TensorEngine.
For each kernel column dj, a banded stationary matrix
    B_dj[k, m] = w[k - m, dj]   (0 <= k - m < 11)
contracts over 128 image rows, while column-shifted slices of the image
slab stream as the moving operand:
    out[m, n] += sum_k B_dj[k, m] * x[r0 + k, n0 + n + dj]
Accumulating the 11 dj-shifted matmuls in one PSUM bank yields the full
11x11 correlation for a [118, 512] output tile.

v2 vs the f32r baseline (138.4us):
- bf16 operands: ~217 ns vs 239.5 ns sustained per 512-col matmul on HW
  (less clock throttle), and input DMA halves. Host casts x -> bf16.
- bf16 output + host cast back to f32: store traffic halves.
- slab 0 is loaded in column chunks so bank 0's matmuls start ~4us in
  instead of waiting ~22us for the whole 2MB slab (subtile DMA deps).
- acts+stores are per [M, 512] bank tile, so the tail after the last
  matmul is one act + one 120KB store instead of a whole-slab store.
"""

import os
import sys

import numpy as np

for _p in ("/opt/trn_rl_repo", "/root/.axon_site/_ro/trn_rl_repo"):
    if os.path.isdir(_p) and _p not in sys.path:
        sys.path.insert(0, _p)

_jp = os.environ.get("JAX_PLATFORMS", "")
if "axon" not in _jp.split(","):
    os.environ["JAX_PLATFORMS"] = ("axon," + _jp).strip(",")

import ml_dtypes
import concourse.bacc as bacc
import concourse.bass as bass
import concourse.mybir as mybir
import concourse.tile as tile
from concourse.bass_utils import run_bass_kernel_spmd

H = W = 4096
KH = KW = 11
OH = OW = H - KH + 1  # 4086
NCORES = 8
ROWS_OUT = 512            # output rows per core
ROWS_IN = ROWS_OUT + KH - 1  # 522
M_FULL = 118              # output rows per full slab (contraction K = 128)
# (x row offset, out row offset, M out rows, band column offset) per slab.
# All slabs contract over K=128 input rows: the 40-row tail reads the last
# 128 slab rows (394..521) and picks the shifted band columns 78..117, so
# the PE always runs at full contraction duty.
SLABS = [(0, 0, 118, 0), (118, 118, 118, 0), (236, 236, 118, 0),
         (354, 354, 118, 0), (394, 472, 40, 78)]
BANK_N = [512] * 7 + [OW - 7 * 512]  # 7x512 + 502 = 4086

_cache: dict = {}
LAST_RESULT = None  # BassKernelResults of the most recent device run


def _build():
    f32 = mybir.dt.float32
    bf16 = mybir.dt.bfloat16
    nc = bacc.Bacc("TRN2", target_bir_lowering=False, debug=False,
                   num_devices=NCORES)
    xs_d = nc.dram_tensor("xs", [ROWS_IN, W], bf16, kind="ExternalInput")
    bd_d = nc.dram_tensor("bands", [128, KW * M_FULL], bf16,
                          kind="ExternalInput")
    bias_d = nc.dram_tensor("biasv", [1, 1], f32, kind="ExternalInput")
    out_d = nc.dram_tensor("out", [ROWS_OUT, OW], bf16, kind="ExternalOutput")

    with tile.TileContext(nc) as tc:
        with (
            tc.tile_pool(name="bp", bufs=1) as bp,
            tc.tile_pool(name="xp", bufs=1) as xp,
            # enough out-tile bufs that an act never waits on the ~2us
            # completion latency of the store 4 banks earlier
            tc.tile_pool(name="op", bufs=14) as op,
            tc.tile_pool(name="pp", bufs=6, space=bass.MemorySpace.PSUM) as pp,
            tc.tile_pool(name="pw", bufs=1, space=bass.MemorySpace.PSUM) as pw,
        ):
            # bands first: single DMA, all 16 SDMA engines, ~1us
            bt = bp.tile([128, KW * M_FULL], bf16, name="bt")
            nc.sync.dma_start(bt[:], bd_d.ap()[:, :])

            # slab 0 in column chunks so bank 0 can start as soon as its
            # columns land (Tile subtile deps): [0:522] then 3 wider chunks
            xt0 = xp.tile([128, W], bf16, tag="xt0", name="xt0")
            for c0, c1 in ((0, 522), (522, 1546), (1546, 2570), (2570, W)):
                nc.sync.dma_start(xt0[:, c0:c1], xs_d.ap()[0:128, c0:c1])

            # remaining slabs: whole-tile DMAs (one per slab, 16 engines)
            xts = {0: xt0}
            for si, (r0, _, _, _) in enumerate(SLABS):
                if si == 0:
                    continue
                xt = xp.tile([128, W], bf16, tag=f"xt{si}", name=f"xt{si}")
                nc.sync.dma_start(xt[:], xs_d.ap()[r0:r0 + 128, :])
                xts[si] = xt

            # bias: one-packet DMA, then broadcast across partitions with a
            # K=1 matmul against a ones row
            bias_sb = bp.tile([1, 1], f32, name="bias_sb")
            nc.sync.dma_start(bias_sb[:], bias_d.ap()[:, :])

            # warm the PE pstate while DMAs land (bands tile is the first
            # DMA to arrive, ~1.5us)
            ones_t = bp.tile([1, 128], f32, name="ones_t")
            nc.gpsimd.memset(ones_t[:], 1.0)
            warm = pw.tile([118, 512], f32, name="warm")
            for i in range(9):
                nc.tensor.matmul(warm[:, :], bt[:, 0:118], bt[:, 0:512],
                                 start=(i == 0), stop=(i == 8))
            bias_ps = pw.tile([128, 1], f32, name="bias_ps")
            nc.tensor.matmul(bias_ps[:], ones_t[:], bias_sb[:],
                             start=True, stop=True)
            bias_bc = bp.tile([128, 1], f32, name="bias_bc")
            nc.scalar.copy(bias_bc[:], bias_ps[:])

            for si, (r0, o0, M, boff) in enumerate(SLABS):
                xt = xts[si]
                for b in range(8):
                    n0 = b * 512
                    N = BANK_N[b]
                    pt = pp.tile([M, 512], f32, tag="ps", name=f"ps{si}_{b}")
                    for dj in range(KW):
                        nc.tensor.matmul(
                            pt[:, :N],
                            bt[:, dj * M_FULL + boff: dj * M_FULL + boff + M],
                            xt[:, n0 + dj: n0 + dj + N],
                            start=(dj == 0),
                            stop=(dj == KW - 1),
                        )
                    ot = op.tile([M, 512], bf16, tag="ot", name=f"ot{si}_{b}")
                    nc.scalar.activation(
                        ot[:, :N], pt[:, :N],
                        mybir.ActivationFunctionType.Identity,
                        bias=bias_bc[0:M, :],
                    )
                    nc.sync.dma_start(out_d.ap()[o0:o0 + M, n0:n0 + N],
                                      ot[:, :N])
    nc.compile()
    return nc


def _bands_from_weight(weight: np.ndarray) -> np.ndarray:
    b = np.zeros((128, KW * M_FULL), np.float32)
    for dj in range(KW):
        col = weight[:, dj].astype(np.float32)
        for m in range(M_FULL):
            b[m:m + KH, dj * M_FULL + m] = col
    return b


def kernel(x: np.ndarray, weight: np.ndarray, bias: np.ndarray,
           _trace: bool = False, **_trace_kwargs) -> np.ndarray:
    global LAST_RESULT
    x = np.asarray(x, dtype=np.float32)
    weight = np.asarray(weight, dtype=np.float32)
    bias_v = np.asarray(bias, dtype=np.float32).reshape(1, 1)

    if "nc" not in _cache:
        _cache["nc"] = _build()
    nc = _cache["nc"]

    xb = x.astype(ml_dtypes.bfloat16)
    bands = _bands_from_weight(weight).astype(ml_dtypes.bfloat16)
    starts = [min(m * ROWS_OUT, H - ROWS_IN) for m in range(NCORES)]
    in_maps = [
        {"xs": np.ascontiguousarray(xb[s:s + ROWS_IN]),
         "bands": bands,
         "biasv": bias_v}
        for s in starts
    ]
    res = run_bass_kernel_spmd(nc, in_maps, core_ids=list(range(NCORES)),
                               trace=_trace, **_trace_kwargs)
    LAST_RESULT = res

    out = np.empty((OH, OW), dtype=np.float32)
    for m, s in enumerate(starts):
        r = np.asarray(res.results[m]["out"], dtype=np.float32)
        g0 = m * ROWS_OUT           # first global output row wanted from core m
        keep0 = g0 - s              # 0 for cores 0-6, 10 for core 7
        take = min(ROWS_OUT - keep0, OH - g0)
        out[g0:g0 + take] = r[keep0:keep0 + take]
    return out


# revision 9
# speedup vs baseline: 1.1694x; 1.1532x over previous
"""Trainium2 Bass kernel: 4096x4096 valid cross-correlation with an 11x11
filter + scalar bias, sharded row-wise across 8 NeuronCores.

Strategy
--------
Host-side sharding (halo = overlapping row slices, no collectives): core m
gets input rows [512m, 512m + 522) (core 7 shifted up to stay in bounds)
and produces output rows [512m, 512m + 512).

Per-core compute: conv expressed as banded matmuls on the TensorEngine.
For each kernel column dj, a banded stationary matrix
    B_dj[k, m] = w[k - m, dj]   (0 <= k - m < 11)
contracts over 128 image rows, while column-shifted slices of the image
slab stream as the moving operand:
    out[m, n] += sum_k B_dj[k, m] * x[r0 + k, n0 + n + dj]
Accumulating the 11 dj-shifted matmuls in one PSUM bank yields the full
11x11 correlation for a [118, 512] output tile.

v2 vs the f32r baseline (138.4us):
- bf16 operands: ~217 ns vs 239.5 ns sustained per 512-col matmul on HW
  (less clock throttle), and input DMA halves. Host casts x -> bf16.
- bf16 output + host cast back to f32: store traffic halves.
- slab 0 is loaded in column chunks so bank 0's matmuls start ~4us in
  instead of waiting ~22us for the whole 2MB slab (subtile DMA deps).
- acts+stores are per [M, 512] bank tile, so the tail after the last
  matmul is one act + one 120KB store instead of a whole-slab store.
"""

import os
import sys

import numpy as np

for _p in ("/opt/trn_rl_repo", "/root/.axon_site/_ro/trn_rl_repo"):
    if os.path.isdir(_p) and _p not in sys.path:
        sys.path.insert(0, _p)

_jp = os.environ.get("JAX_PLATFORMS", "")
if "axon" not in _jp.split(","):
    os.environ["JAX_PLATFORMS"] = ("axon," + _jp).strip(",")

import ml_dtypes
import concourse.bacc as bacc
import concourse.bass as bass
import concourse.mybir as mybir
import concourse.tile as tile
from concourse.bass_utils import run_bass_kernel_spmd

H = W = 4096
KH = KW = 11
OH = OW = H - KH + 1  # 4086
NCORES = 8
ROWS_OUT = 512            # output rows per core
ROWS_IN = ROWS_OUT + KH - 1  # 522
M_FULL = 118              # output rows per full slab (contraction K = 128)
# (x row offset, out row offset, M out rows, band column offset) per slab.
# All slabs contract over K=128 input rows: the 40-row tail reads the last
# 128 slab rows (394..521) and picks the shifted band columns 78..117, so
# the PE always runs at full contraction duty.
SLABS = [(0, 0, 118, 0), (118, 118, 118, 0), (236, 236, 118, 0),
         (354, 354, 118, 0), (394, 472, 40, 78)]
BANK_N = [512] * 7 + [OW - 7 * 512]  # 7x512 + 502 = 4086

_cache: dict = {}
LAST_RESULT = None  # BassKernelResults of the most recent device run


def _build():
    f32 = mybir.dt.float32
    bf16 = mybir.dt.bfloat16
    nc = bacc.Bacc("TRN2", target_bir_lowering=False, debug=False,
                   num_devices=NCORES)
    xs_d = nc.dram_tensor("xs", [ROWS_IN, W], bf16, kind="ExternalInput")
    bd_d = nc.dram_tensor("bands", [128, KW * M_FULL], bf16,
                          kind="ExternalInput")
    bias_d = nc.dram_tensor("biasv", [1, 1], f32, kind="ExternalInput")
    out_d = nc.dram_tensor("out", [ROWS_OUT, OW], bf16, kind="ExternalOutput")

    with tile.TileContext(nc) as tc:
        with (
            tc.tile_pool(name="bp", bufs=1) as bp,
            tc.tile_pool(name="xp", bufs=1) as xp,
            tc.tile_pool(name="op", bufs=3) as op,
            tc.tile_pool(name="pp", bufs=6, space=bass.MemorySpace.PSUM) as pp,
            tc.tile_pool(name="pw", bufs=1, space=bass.MemorySpace.PSUM) as pw,
        ):
            # slab 0 in column chunks so bank 0 can start as soon as its
            # columns land (Tile subtile deps); bank 0's chunk goes first,
            # bands second, so the first matmul group is ready ~2us in
            xt0 = xp.tile([128, W], bf16, tag="xt0", name="xt0")
            bt = bp.tile([128, KW * M_FULL], bf16, name="bt")
            nc.sync.dma_start(xt0[:, 0:522], xs_d.ap()[0:128, 0:522])
            nc.sync.dma_start(bt[:], bd_d.ap()[:, :])
            for c0, c1 in ((522, 1546), (1546, 2570), (2570, W)):
                nc.sync.dma_start(xt0[:, c0:c1], xs_d.ap()[0:128, c0:c1])

            # remaining slabs: whole-tile DMAs (one per slab, 16 engines)
            xts = {0: xt0}
            for si, (r0, _, _, _) in enumerate(SLABS):
                if si == 0:
                    continue
                xt = xp.tile([128, W], bf16, tag=f"xt{si}", name=f"xt{si}")
                nc.sync.dma_start(xt[:], xs_d.ap()[r0:r0 + 128, :])
                xts[si] = xt

            # bias: one-packet DMA, then broadcast across partitions with a
            # K=1 matmul against a ones row
            bias_sb = bp.tile([1, 1], f32, name="bias_sb")
            nc.sync.dma_start(bias_sb[:], bias_d.ap()[:, :])

            # warm the PE pstate while DMAs land; short-N matmuls on a
            # memset tile have no DMA dependency, so ramping starts at t=0
            ones_t = bp.tile([1, 128], f32, name="ones_t")
            nc.gpsimd.memset(ones_t[:], 1.0)
            warm_src = bp.tile([128, 128], bf16, name="warm_src")
            nc.gpsimd.memset(warm_src[:], 1.0)
            warm = pw.tile([118, 512], f32, name="warm")
            for i in range(16):
                nc.tensor.matmul(warm[:, 0:128], warm_src[:, 0:118],
                                 warm_src[:, 0:128],
                                 start=(i == 0), stop=(i == 15))
            bias_ps = pw.tile([128, 1], f32, name="bias_ps")
            nc.tensor.matmul(bias_ps[:], ones_t[:], bias_sb[:],
                             start=True, stop=True)
            bias_bc = bp.tile([128, 1], f32, name="bias_bc")
            nc.scalar.copy(bias_bc[:], bias_ps[:])

            # acts per bank, but one wide store per slab: a [M, 4086] bf16
            # store has 8KB rows (good per-engine DMA efficiency) and only
            # 5 stores pay the ~2us HBM-write completion latency
            for si, (r0, o0, M, boff) in enumerate(SLABS):
                xt = xts[si]
                ot = op.tile([M, OW], bf16, tag="ot", name=f"ot{si}")
                for b in range(8):
                    n0 = b * 512
                    N = BANK_N[b]
                    pt = pp.tile([M, 512], f32, tag="ps", name=f"ps{si}_{b}")
                    for dj in range(KW):
                        nc.tensor.matmul(
                            pt[:, :N],
                            bt[:, dj * M_FULL + boff: dj * M_FULL + boff + M],
                            xt[:, n0 + dj: n0 + dj + N],
                            start=(dj == 0),
                            stop=(dj == KW - 1),
                        )
                    nc.scalar.activation(
                        ot[:, n0:n0 + N], pt[:, :N],
                        mybir.ActivationFunctionType.Identity,
                        bias=bias_bc[0:M, :],
                    )
                nc.sync.dma_start(out_d.ap()[o0:o0 + M, :], ot[:])
    nc.compile()
    return nc


def _bands_from_weight(weight: np.ndarray) -> np.ndarray:
    b = np.zeros((128, KW * M_FULL), np.float32)
    for dj in range(KW):
        col = weight[:, dj].astype(np.float32)
        for m in range(M_FULL):
            b[m:m + KH, dj * M_FULL + m] = col
    return b


def kernel(x: np.ndarray, weight: np.ndarray, bias: np.ndarray,
           _trace: bool = False, **_trace_kwargs) -> np.ndarray:
    global LAST_RESULT
    x = np.asarray(x, dtype=np.float32)
    weight = np.asarray(weight, dtype=np.float32)
    bias_v = np.asarray(bias, dtype=np.float32).reshape(1, 1)

    if "nc" not in _cache:
        _cache["nc"] = _build()
    nc = _cache["nc"]

    xb = x.astype(ml_dtypes.bfloat16)
    bands = _bands_from_weight(weight).astype(ml_dtypes.bfloat16)
    starts = [min(m * ROWS_OUT, H - ROWS_IN) for m in range(NCORES)]
    in_maps = [
        {"xs": np.ascontiguousarray(xb[s:s + ROWS_IN]),
         "bands": bands,
         "biasv": bias_v}
        for s in starts
    ]
    res = run_bass_kernel_spmd(nc, in_maps, core_ids=list(range(NCORES)),
                               trace=_trace, **_trace_kwargs)
    LAST_RESULT = res

    out = np.empty((OH, OW), dtype=np.float32)
    for m, s in enumerate(starts):
        r = np.asarray(res.results[m]["out"], dtype=np.float32)
        g0 = m * ROWS_OUT           # first global output row wanted from core m
        keep0 = g0 - s              # 0 for cores 0-6, 10 for core 7
        take = min(ROWS_OUT - keep0, OH - g0)
        out[g0:g0 + take] = r[keep0:keep0 + take]
    return out
